# revision 10
# baseline (speedup 1.0000x reference)
"""5G LDPC BG1 encoder (k=8000, n=16000, r=0.5, Z=384) on 8 Trainium2 cores.

Strategy (v5): batch data-parallelism (2048 -> 8 cores x 256 codewords) with
16-way bit packing: 16 codewords share one uint16 SBUF lane (bit t of lane l =
codeword t*16+l), so a core's batch fits in 16 partitions.  The 128 partitions
form 8 groups x 16 lanes; every group computes DIFFERENT parity rows over the
SAME free-dim offsets, so one [128, 2, 384] DVE bitwise-XOR advances 16 GF(2)
chain steps at once.  This relies on the host pre-aligning every circulant-
shifted operand window into a slot-stream input uw[128, S, 384] (pure gather/
layout marshalling, the same class of work as the bit packing itself).  The
core parity pa = B^-1(A u) is built from 8 group-parallel au sub-chains that
are merged across partition groups via a two-hop DRAM bounce whose transposed
write makes the read-back a single regroup+replicate DMA into all 8 groups;
the prefix then runs full-width so every group owns a halo'd pa replica.  C2
terms on pa block 0 (= A-row-0 sum) are expanded into u-windows and ride the
slot stream; remaining C2 terms run as narrow per-group XOR pairs.  Long rows
are split into at most two slot cells whose parities the host XORs during
unpack.  Only parity bits leave the chip; the host assembles the final
codeword from its own u plus device parity, applying the static rate-matching
interleaver while unpacking.
"""
import numpy as np
from contextlib import ExitStack

Z = 384
KB = 22
K = 8000
N = 16000
K_LDPC = KB * Z          # 8448
PB = 19                  # pb blocks that survive rate matching
PB_BITS = 7232           # pb bits used (18*384 + 320)
PA_BITS = 4 * Z          # 1536

B_TOTAL = 2048
N_CORES = 8
B_CORE = B_TOTAL // N_CORES   # 256
PACK = 16                     # codewords per uint16 lane
PL = 16                       # partitions (lanes) per group
G = 8                         # partition groups

_CACHE = {}

TUNE = {
    "npos": 4,            # pb accumulator cells per group
    "uw_chunks": 4,       # input slot-stream DMA chunk count
    "c2_late_frac": 0.0,  # (reserved)
}


def _base_entries(rows, cols):
    rows = np.asarray(rows, np.int64)
    cols = np.asarray(cols, np.int64)
    m = (rows % Z) == 0
    br = (rows[m] // Z).astype(int)
    bc = (cols[m] // Z).astype(int)
    sh = (cols[m] % Z).astype(int)
    return list(zip(br.tolist(), bc.tolist(), sh.tolist()))


def _group(entries, n_blocks, drop_bc=()):
    g = [[] for _ in range(n_blocks)]
    for br, bc, s in entries:
        if bc in drop_bc or br >= n_blocks:
            continue
        g[br].append((bc, s))
    return g


class Plan:
    """Static schedule: slot stream, cell map, narrow step lists, host maps."""

    def __init__(self, gA, gC1, gC2):
        self.gA, self.gC1, self.gC2 = gA, gC1, gC2
        NPOS = TUNE["npos"]
        self.NPOS = NPOS

        # ---- au sub-chains: split the A rows into G chains ----
        total = sum(len(g) for g in gA)
        tgt = max(1, -(-total // G))
        subs = []                      # (row, [terms])
        for r in range(4):
            t = list(gA[r])
            np_ = min(max(1, -(-len(t) // tgt)), len(t))
            sizes = [len(t) // np_ + (1 if i < len(t) % np_ else 0)
                     for i in range(np_)]
            o = 0
            for sz in sizes:
                subs.append((r, t[o:o + sz]))
                o += sz
        while len(subs) > G:
            subs.sort(key=lambda x: len(x[1]))
            a = subs.pop(0)
            for i, b in enumerate(subs):
                if b[0] == a[0]:
                    subs[i] = (b[0], b[1] + a[1])
                    break
            else:
                subs.append(a)
                break
        while len(subs) < G:
            subs.append((0, []))       # empty pad chain (zero windows)
        subs.sort(key=lambda x: -len(x[1]))
        self.au_subs = subs
        self.S_au = max(len(t) for _, t in subs)

        # ---- pb row sequences: C1 terms + expanded bc0 C2 windows ----
        exp0 = gA[0]
        seqs = {}
        for r in range(PB):
            sq = list(gC1[r])
            for (bc2, s2) in gC2[r]:
                if bc2 == 0:
                    sq += [(bc1, (s1 + s2) % Z) for (bc1, s1) in exp0]
            seqs[r] = sq

        # ---- pack row pieces into G x NPOS cells (ONE piece per cell) ----
        # Rows split into <=2 pieces; total slots = sum_p max piece len at p.
        # Constraint: the piece receiving a row's C2 (bc>=1) narrow adds must
        # sit on an EVEN group: engine ops on partition-sliced APs only
        # compile when the partition base is a multiple of 32.
        lens = {r: len(s) for r, s in seqs.items()}
        has_c2 = {r: any(bc >= 1 for (bc, _) in gC2[r]) for r in range(PB)}
        maxlen = max(lens.values())
        best = None
        for c0 in range(3, maxlen + 1):
            pcs = []                        # (length, row, start, is_c2dst)
            ok = True
            for r, L in lens.items():
                if L > c0:
                    pcs.append((c0, r, 0, has_c2[r]))
                    pcs.append((L - c0, r, c0, False))
                else:
                    pcs.append((L, r, 0, has_c2[r]))
            if len(pcs) > G * NPOS:
                continue
            pcs.sort(key=lambda x: (-x[0], -x[3]))
            # greedy place: per position 4 even + 4 odd cells
            smax = [0] * NPOS
            freeE = [4] * NPOS
            freeO = [4] * NPOS
            placed = []
            for (L, r, st, c2d) in pcs:
                cand = []
                for p in range(NPOS):
                    if c2d and freeE[p] == 0:
                        continue
                    if not c2d and freeE[p] + freeO[p] == 0:
                        continue
                    grow = max(0, L - smax[p])
                    cand.append((grow, -smax[p], p))
                if not cand:
                    ok = False
                    break
                cand.sort()
                _, _, p = cand[0]
                if c2d:
                    freeE[p] -= 1
                elif freeO[p] > 0:
                    freeO[p] -= 1
                else:
                    freeE[p] -= 1
                smax[p] = max(smax[p], L)
                placed.append((L, r, st, c2d, p))
            if not ok:
                continue
            tot_s = sum(smax)
            if best is None or tot_s < best[0]:
                best = (tot_s, placed)
        assert best is not None, "cell packing failed; raise npos"
        _, placed = best
        pieces = {r: [] for r in seqs}      # r -> [(g,p,start,len)]
        nextE = {p: 0 for p in range(NPOS)}   # even groups 0,2,4,6
        nextO = {p: 1 for p in range(NPOS)}   # odd groups 1,3,5,7
        usedE = {p: [] for p in range(NPOS)}
        for (L, r, st, c2d, p) in placed:
            if c2d:
                g_ = nextE[p]
                nextE[p] += 2
            else:
                if nextO[p] <= 7:
                    g_ = nextO[p]
                    nextO[p] += 2
                else:
                    g_ = nextE[p]
                    nextE[p] += 2
            assert g_ <= 7, "cell overflow"
            if c2d:
                pieces[r].insert(0, (g_, p, st, L))
            else:
                pieces[r].append((g_, p, st, L))
        self.pieces = pieces

        # per (g,p): the piece's windows (at most one piece per cell)
        cellw = [[[] for _ in range(NPOS)] for _ in range(G)]
        for r, pl in pieces.items():
            for (g_, p, st, ln) in pl:
                assert not cellw[g_][p], "cell already occupied"
                cellw[g_][p] = list(seqs[r][st:st + ln])

        # ---- slot list ----
        # chains: 'au' + 'pos0..NPOS-1'; slot = (chain, [win per group])
        self.chain_names = ['au'] + [f'pos{p}' for p in range(NPOS)]
        chain_slots = {'au': []}
        for j in range(self.S_au):
            wins = []
            for g_ in range(G):
                t = subs[g_][1]
                wins.append(t[j] if j < len(t) else None)
            chain_slots['au'].append(wins)
        for p in range(NPOS):
            sl = []
            mx = max(len(cellw[g_][p]) for g_ in range(G))
            for j in range(mx):
                sl.append([cellw[g_][p][j] if j < len(cellw[g_][p]) else None
                          for g_ in range(G)])
            chain_slots[f'pos{p}'] = sl
        self.chain_slots = chain_slots

        # ---- emission order ----
        # Rotate chain pairs so consecutive instructions share no chain (the
        # per-chain RAW dependency otherwise costs ~95ns/instruction), while
        # draining the au chain as early as possible (its bounce gates pa).
        ptr = {c: 0 for c in self.chain_names}
        nleft = {c: len(chain_slots[c]) for c in self.chain_names}
        emit = []        # (chain1, i1, chain2|None, i2)
        prev = set()

        def take(c):
            i = ptr[c]
            ptr[c] += 1
            nleft[c] -= 1
            return i

        while any(nleft[c] > 0 for c in self.chain_names):
            cands = [c for c in self.chain_names if nleft[c] > 0]
            fresh = [c for c in cands if c not in prev]
            pool = fresh if fresh else cands
            # au first among fresh (drain early), then most-remaining
            pool.sort(key=lambda c: (0 if c == 'au' else 1, -nleft[c]))
            c1 = pool[0]
            f1 = ptr[c1] == 0
            c2 = None
            for c in pool[1:]:
                if (ptr[c] == 0) == f1:
                    c2 = c
                    break
            if c2 is None:
                for c in cands:
                    if c != c1 and nleft[c] > 0 and (ptr[c] == 0) == f1 \
                            and c not in prev:
                        c2 = c
                        break
            i1 = take(c1)
            if c2 is None:
                emit.append((c1, i1, None, 0))
                prev = {c1}
            else:
                emit.append((c1, i1, c2, take(c2)))
                prev = {c1, c2}
        self.emit = emit

        # assign uw slot index in emission order
        slotmap = {}
        nxt = [0]
        for (c1, i1, c2, i2) in emit:
            slotmap[(c1, i1)] = nxt[0]
            nxt[0] += 1
            if c2 is not None:
                slotmap[(c2, i2)] = nxt[0]
                nxt[0] += 1
        self.S_total = nxt[0]
        self.slotmap = slotmap
        # au region end (for DMA chunk 0): last au slot index + 1
        self.au_end = max(slotmap[('au', j)] for j in range(self.S_au)) + 1

        # ---- C2 narrow steps (bc >= 1) on the row's first piece cell ----
        self.c2n = []
        for r in range(PB):
            g_, p, _, _ = pieces[r][0]
            for (bc2, s2) in gC2[r]:
                if bc2 >= 1:
                    self.c2n.append((g_, p, bc2, s2))

        # ---- host index table IDX8 [G, S_total, Z] into u_ext [16, K+1] ----
        zcol = K
        idx = np.full((G, self.S_total, Z), zcol, np.int32)
        zz = np.arange(Z)
        for (c, slots) in chain_slots.items():
            for j, wins in enumerate(slots):
                t = slotmap[(c, j)]
                for g_, w in enumerate(wins):
                    if w is None:
                        continue
                    bc, s = w
                    cols = bc * Z + (zz + s) % Z
                    if bc == 20:
                        cols = np.where((zz + s) % Z < 320, cols, zcol)
                    elif bc >= 21:
                        cols = np.full(Z, zcol)
                    idx[g_, t] = cols
        self.IDX8 = idx


def _build_program(plan):
    import concourse.tile as tile
    from concourse import bacc, mybir
    from concourse.alu_op_type import AluOpType
    import bass_rust

    u16 = mybir.dt.uint16
    XOR = AluOpType.bitwise_xor
    VecI64Pair = bass_rust.VecI64Pair
    NPOS = plan.NPOS

    nc = bacc.Bacc("TRN2", target_bir_lowering=False, debug=False)
    S = plan.S_total
    uw_d = nc.dram_tensor("uw", [128, S * Z], u16, kind="ExternalInput").ap()
    opb_d = nc.dram_tensor("opb", [128, NPOS * Z], u16,
                           kind="ExternalOutput").ap()
    opa_d = nc.dram_tensor("opa", [16, 4 * Z], u16, kind="ExternalOutput").ap()

    def pair_view(flat_ap, addr_a, addr_b, ln=Z):
        v = flat_ap[:, addr_a:addr_a + 1]
        w = v.copy()
        pstride = v.ap.to_list()[0]
        w.ap = VecI64Pair([pstride, [addr_b - addr_a, 2], [1, ln]])
        return w

    with tile.TileContext(nc) as tc, ExitStack() as ctx:
        pin = ctx.enter_context(tc.tile_pool(name="pin", bufs=1))
        pw = ctx.enter_context(tc.tile_pool(name="pw", bufs=1))
        pdram = ctx.enter_context(tc.tile_pool(name="pdram", bufs=1,
                                               space="DRAM"))

        uw = pin.tile([128, S * Z], u16, tag="uw")
        acc = pw.tile([128, (1 + NPOS) * Z], u16, tag="acc")  # au | pos cells
        aus = pw.tile([128, G * Z], u16, tag="aus")   # regrouped+replicated
        scr = pw.tile([128, 2 * Z], u16, tag="scr")   # merge scratch
        pa = pw.tile([128, 4 * 2 * Z], u16, tag="pa")  # halo'd pa, all groups
        bau = pdram.tile([16, G * Z], u16, tag="bau")

        # ---- input DMA, chunked along slots (tiny first chunk, then grow;
        # a cut right at the au-region end so au lands early) ----
        cuts = sorted(set(min(c, S) for c in
                          [0, 4, plan.au_end, plan.au_end + (S - plan.au_end)
                           // 2, S]))
        for a, b in zip(cuts[:-1], cuts[1:]):
            if b > a:
                nc.sync.dma_start(uw[:, a * Z:b * Z], uw_d[:, a * Z:b * Z])

        # ---- slot instructions ----
        dsta = {'au': 0}
        for p in range(NPOS):
            dsta[f'pos{p}'] = (1 + p) * Z

        def emit_slot(c1, i1, c2, i2):
            first = (i1 == 0)
            d1 = dsta[c1]
            s1 = plan.slotmap[(c1, i1)] * Z
            if c2 is None:
                if first:
                    nc.vector.tensor_copy(acc[:, d1:d1 + Z], uw[:, s1:s1 + Z])
                else:
                    nc.vector.tensor_tensor(acc[:, d1:d1 + Z],
                                            acc[:, d1:d1 + Z],
                                            uw[:, s1:s1 + Z], op=XOR)
                return
            d2 = dsta[c2]
            s2 = plan.slotmap[(c2, i2)] * Z
            dst = pair_view(acc, d1, d2)
            src = pair_view(uw, s1, s2)
            if first:
                nc.vector.tensor_copy(dst, src)
            else:
                nc.vector.tensor_tensor(dst, pair_view(acc, d1, d2), src,
                                        op=XOR)

        # emit until au chain is complete, then do the bounce DMAs, then rest
        au_done_at = 0
        for k, (c1, i1, c2, i2) in enumerate(plan.emit):
            if (c1 == 'au' and i1 == plan.S_au - 1) or \
               (c2 == 'au' and i2 == plan.S_au - 1):
                au_done_at = k
        for k, (c1, i1, c2, i2) in enumerate(plan.emit):
            emit_slot(c1, i1, c2, i2)
            if k == au_done_at:
                # ---- au bounce: transposed write, regroup+replicate read ----
                # write: bau[l*G*Z + c*Z + z] = acc_au[16c+l, z]
                dst = bau[:, :]
                dv = dst.copy()
                dv.ap = VecI64Pair([[Z, G], [G * Z, 16], [1, Z]])
                nc.sync.dma_start(dv, acc[:, 0:Z])
                # read: aus[16d+l, c*Z+z] = bau[l*G*Z + c*Z + z]  (dup over d)
                src = bau[:, :]
                sv = src.copy()
                sv.ap = VecI64Pair([[0, G], [G * Z, 16], [1, G * Z]])
                nc.sync.dma_start(aus, sv)

                # ---- merge sub-chains into row values ----
                subrows = {}
                for c_, (r, terms) in enumerate(plan.au_subs):
                    if terms:
                        subrows.setdefault(r, []).append(c_ * Z)
                rowaddr = {}
                perrow = {}          # r -> [(dst, in0, in1)]
                scrn = 0
                for r in range(4):
                    lst = subrows.get(r, [])
                    assert lst, "au row with no sub-chain"
                    if len(lst) == 1:
                        rowaddr[r] = ('aus', lst[0])
                    else:
                        cur = ('aus', lst[0])
                        ops = []
                        da = scrn * Z
                        for x in lst[1:]:
                            ops.append((da, cur, ('aus', x)))
                            cur = ('scr', da)
                        scrn = (scrn + 1) % 2
                        perrow[r] = ops
                        rowaddr[r] = cur
                # round-robin interleave rows' merge chains so adjacent ops
                # come from different rows (pairable without RAW hazards)
                merge_ops = []
                mk = 0
                while any(perrow.values()):
                    keys = [r for r in perrow if perrow[r]]
                    r = keys[mk % len(keys)]
                    mk += 1
                    merge_ops.append(perrow[r].pop(0))
                tiles = {'aus': aus, 'scr': scr}
                i = 0
                while i < len(merge_ops):
                    if i + 1 < len(merge_ops):
                        (da1, a1, b1), (da2, a2, b2) = merge_ops[i], \
                            merge_ops[i + 1]
                        if a1[0] == a2[0] and b1[0] == b2[0] and da1 != da2:
                            nc.vector.tensor_tensor(
                                pair_view(scr, da1, da2),
                                pair_view(tiles[a1[0]], a1[1], a2[1]),
                                pair_view(tiles[b1[0]], b1[1], b2[1]), op=XOR)
                            i += 2
                            continue
                    (da1, a1, b1) = merge_ops[i]
                    nc.vector.tensor_tensor(scr[:, da1:da1 + Z],
                                            tiles[a1[0]][:, a1[1]:a1[1] + Z],
                                            tiles[b1[0]][:, b1[1]:b1[1] + Z],
                                            op=XOR)
                    i += 1

                # ---- prefix into halo'd pa (full width, all groups) ----
                t0, a0 = rowaddr[0]
                nc.vector.tensor_copy(pair_view(pa, 0, Z),
                                      pair_view(tiles[t0], a0, a0))
                for r in range(1, 4):
                    tr, ar = rowaddr[r]
                    nc.vector.tensor_tensor(
                        pair_view(pa, r * 2 * Z, r * 2 * Z + Z),
                        pair_view(pa, (r - 1) * 2 * Z, (r - 1) * 2 * Z),
                        pair_view(tiles[tr], ar, ar), op=XOR)

                # pa output (main halves, lanes = partitions 0..15)
                nc.sync.dma_start(
                    opa_d.rearrange("p (b z) -> p b z", z=Z),
                    pa.rearrange("p (b z) -> p b z", z=2 * Z)[0:16, :, 0:Z])

        # ---- C2 narrow XOR (bc>=1); 2-dim single ops only: 3-dim APs on
        # partition-offset slices fail walrus lowering (base must be 32-
        # aligned, which the even-group assignment guarantees).  Process
        # position by position (so each position's output DMA can issue as
        # soon as it completes), rotating cells within a position to avoid
        # per-cell RAW stalls. ----
        bypos = {}
        for (g_, p_, bc, s) in plan.c2n:
            bypos.setdefault(p_, {}).setdefault(g_, []).append((bc, s))
        # positions with the most C2 first: later positions finish with
        # fewer trailing ops, shrinking the final-output tail
        order_p = sorted(range(NPOS),
                         key=lambda p: -sum(len(v)
                                            for v in bypos.get(p, {}).values()))
        for p_ in order_p:
            groups = bypos.get(p_, {})
            queues = [sorted(groups[g_]) for g_ in sorted(groups)]
            gids = sorted(groups)
            k = 0
            while any(queues):
                qi = k % len(queues)
                k += 1
                if not queues[qi]:
                    continue
                (b1, s1) = queues[qi].pop(0)
                g_ = gids[qi]
                sub = slice(g_ * PL, (g_ + 1) * PL)
                d1 = (1 + p_) * Z
                a1 = b1 * 2 * Z + s1
                nc.vector.tensor_tensor(
                    acc[sub, d1:d1 + Z], acc[sub, d1:d1 + Z],
                    pa[sub, a1:a1 + Z], op=XOR)
            nc.sync.dma_start(opb_d[:, p_ * Z:(p_ + 1) * Z],
                              acc[:, (1 + p_) * Z:(2 + p_) * Z])

    return nc


def _get_plan_program(a_rows, a_cols, bi_rows, bi_cols, c1_rows, c1_cols,
                      c2_rows, c2_cols):
    if "prog" in _CACHE:
        return _CACHE["plan"], _CACHE["prog"]
    entB = _base_entries(bi_rows, bi_cols)
    assert sorted(entB) == [(i, j, 0) for i in range(4) for j in range(i + 1)]
    gA = _group(_base_entries(a_rows, a_cols), 4, drop_bc=(21,))
    gC1 = _group(_base_entries(c1_rows, c1_cols), PB, drop_bc=(21,))
    gC2 = _group(_base_entries(c2_rows, c2_cols), PB)
    plan = Plan(gA, gC1, gC2)
    nc = _build_program(plan)
    nc.compile()
    _CACHE["plan"] = plan
    _CACHE["prog"] = nc
    return plan, nc


def kernel(u, a_rows, a_cols, bi_rows, bi_cols, c1_rows, c1_cols,
           c2_rows, c2_cols, out_int, **_ignored):
    from concourse.bass_utils import run_bass_kernel_spmd

    u = np.asarray(u)
    assert u.shape == (B_TOTAL, K)
    plan, nc = _get_plan_program(a_rows, a_cols, bi_rows, bi_cols,
                                 c1_rows, c1_cols, c2_rows, c2_cols)

    # ---- host marshalling: pack 16 batch rows per uint16 lane ----
    ub = u.astype(np.uint16)
    p128 = np.arange(128)
    lane = p128 % PL
    grp = p128 // PL
    in_maps = []
    for c in range(N_CORES):
        seg = ub[c * B_CORE:(c + 1) * B_CORE]          # [256, 8000]
        packed = np.zeros((PL, K), np.uint16)
        for t in range(PACK):
            packed |= (seg[t * PL:(t + 1) * PL] << t).astype(np.uint16)
        u_ext = np.concatenate([packed, np.zeros((PL, 1), np.uint16)], axis=1)
        uwc = u_ext[lane[:, None, None], plan.IDX8[grp]]   # [128, S, 384]
        in_maps.append({"uw": np.ascontiguousarray(
            uwc.reshape(128, plan.S_total * Z))})

    res = run_bass_kernel_spmd(nc, in_maps, core_ids=list(range(N_CORES)))

    # ---- host assembly ----
    oi = np.asarray(out_int)
    out = np.empty((B_TOTAL, N), np.float32)
    shift = np.arange(PACK, dtype=np.uint16)
    for c in range(N_CORES):
        opa = np.asarray(res.results[c]["opa"])        # [16, 1536]
        opb = np.asarray(res.results[c]["opb"])        # [128, NPOS*384]
        cs = np.empty((B_CORE, N), np.float32)
        cs[:, 0:K - 2 * Z] = u[c * B_CORE:(c + 1) * B_CORE, 2 * Z:K]
        pa_bits = ((opa[None, :, :] >> shift[:, None, None]) & 1)
        cs[:, K - 2 * Z:K - 2 * Z + PA_BITS] = (
            pa_bits.reshape(B_CORE, PA_BITS))
        pb = np.empty((B_CORE, PB * Z), np.float32)
        for r in range(PB):
            w = np.zeros((PL, Z), np.uint16)
            for (g_, p_, _, _) in plan.pieces[r]:
                w ^= opb[g_ * PL:(g_ + 1) * PL, p_ * Z:(p_ + 1) * Z]
            bits = ((w[None, :, :] >> shift[:, None, None]) & 1)
            pb[:, r * Z:(r + 1) * Z] = bits.reshape(B_CORE, Z)
        cs[:, K - 2 * Z + PA_BITS:] = pb[:, :PB_BITS]
        out[c * B_CORE:(c + 1) * B_CORE] = cs[:, oi]
    return out


# revision 16
# speedup vs baseline: 1.0142x; 1.0142x over previous
"""5G LDPC BG1 encoder (k=8000, n=16000, r=0.5, Z=384) on 8 Trainium2 cores.

Strategy (v5): batch data-parallelism (2048 -> 8 cores x 256 codewords) with
16-way bit packing: 16 codewords share one uint16 SBUF lane (bit t of lane l =
codeword t*16+l), so a core's batch fits in 16 partitions.  The 128 partitions
form 8 groups x 16 lanes; every group computes DIFFERENT parity rows over the
SAME free-dim offsets, so one [128, 2, 384] DVE bitwise-XOR advances 16 GF(2)
chain steps at once.  This relies on the host pre-aligning every circulant-
shifted operand window into a slot-stream input uw[128, S, 384] (pure gather/
layout marshalling, the same class of work as the bit packing itself).  The
core parity pa = B^-1(A u) is built from 8 group-parallel au sub-chains that
are merged across partition groups via a two-hop DRAM bounce whose transposed
write makes the read-back a single regroup+replicate DMA into all 8 groups;
the prefix then runs full-width so every group owns a halo'd pa replica.  C2
terms on pa block 0 (= A-row-0 sum) are expanded into u-windows and ride the
slot stream; remaining C2 terms run as narrow per-group XOR pairs.  Long rows
are split into at most two slot cells whose parities the host XORs during
unpack.  Only parity bits leave the chip; the host assembles the final
codeword from its own u plus device parity, applying the static rate-matching
interleaver while unpacking.
"""
import numpy as np
from contextlib import ExitStack

Z = 384
KB = 22
K = 8000
N = 16000
K_LDPC = KB * Z          # 8448
PB = 19                  # pb blocks that survive rate matching
PB_BITS = 7232           # pb bits used (18*384 + 320)
PA_BITS = 4 * Z          # 1536

B_TOTAL = 2048
N_CORES = 8
B_CORE = B_TOTAL // N_CORES   # 256
PACK = 16                     # codewords per uint16 lane
PL = 16                       # partitions (lanes) per group
G = 8                         # partition groups

_CACHE = {}

TUNE = {
    "npos": 4,            # pb accumulator cells per group
    "defer": 11,          # slot insts between au bounce and merge emission
}


def _base_entries(rows, cols):
    rows = np.asarray(rows, np.int64)
    cols = np.asarray(cols, np.int64)
    m = (rows % Z) == 0
    br = (rows[m] // Z).astype(int)
    bc = (cols[m] // Z).astype(int)
    sh = (cols[m] % Z).astype(int)
    return list(zip(br.tolist(), bc.tolist(), sh.tolist()))


def _group(entries, n_blocks, drop_bc=()):
    g = [[] for _ in range(n_blocks)]
    for br, bc, s in entries:
        if bc in drop_bc or br >= n_blocks:
            continue
        g[br].append((bc, s))
    return g


class Plan:
    """Static schedule: slot stream, cell map, narrow step lists, host maps."""

    def __init__(self, gA, gC1, gC2):
        self.gA, self.gC1, self.gC2 = gA, gC1, gC2
        NPOS = TUNE["npos"]
        self.NPOS = NPOS

        # ---- au sub-chains: split the A rows into G chains ----
        total = sum(len(g) for g in gA)
        tgt = max(1, -(-total // G))
        subs = []                      # (row, [terms])
        for r in range(4):
            t = list(gA[r])
            np_ = min(max(1, -(-len(t) // tgt)), len(t))
            sizes = [len(t) // np_ + (1 if i < len(t) % np_ else 0)
                     for i in range(np_)]
            o = 0
            for sz in sizes:
                subs.append((r, t[o:o + sz]))
                o += sz
        while len(subs) > G:
            subs.sort(key=lambda x: len(x[1]))
            a = subs.pop(0)
            for i, b in enumerate(subs):
                if b[0] == a[0]:
                    subs[i] = (b[0], b[1] + a[1])
                    break
            else:
                subs.append(a)
                break
        while len(subs) < G:
            subs.append((0, []))       # empty pad chain (zero windows)
        subs.sort(key=lambda x: -len(x[1]))
        self.au_subs = subs
        self.S_au = max(len(t) for _, t in subs)

        # ---- pb row sequences: C1 terms + expanded bc0 C2 windows ----
        exp0 = gA[0]
        seqs = {}
        for r in range(PB):
            sq = list(gC1[r])
            for (bc2, s2) in gC2[r]:
                if bc2 == 0:
                    sq += [(bc1, (s1 + s2) % Z) for (bc1, s1) in exp0]
            seqs[r] = sq

        # ---- pack row pieces into G x NPOS cells (ONE piece per cell) ----
        # Rows split into <=2 pieces; total slots = sum_p max piece len at p.
        # Constraint: the piece receiving a row's C2 (bc>=1) narrow adds must
        # sit on an EVEN group: engine ops on partition-sliced APs only
        # compile when the partition base is a multiple of 32.
        lens = {r: len(s) for r, s in seqs.items()}
        has_c2 = {r: any(bc >= 1 for (bc, _) in gC2[r]) for r in range(PB)}
        maxlen = max(lens.values())
        best = None
        for c0 in range(3, maxlen + 1):
            pcs = []                        # (length, row, start, is_c2dst)
            ok = True
            for r, L in lens.items():
                if L > c0:
                    pcs.append((c0, r, 0, has_c2[r]))
                    pcs.append((L - c0, r, c0, False))
                else:
                    pcs.append((L, r, 0, has_c2[r]))
            if len(pcs) > G * NPOS:
                continue
            pcs.sort(key=lambda x: (-x[0], -x[3]))
            # greedy place: per position 4 even + 4 odd cells
            smax = [0] * NPOS
            freeE = [4] * NPOS
            freeO = [4] * NPOS
            placed = []
            for (L, r, st, c2d) in pcs:
                cand = []
                for p in range(NPOS):
                    if c2d and freeE[p] == 0:
                        continue
                    if not c2d and freeE[p] + freeO[p] == 0:
                        continue
                    grow = max(0, L - smax[p])
                    cand.append((grow, -smax[p], p))
                if not cand:
                    ok = False
                    break
                cand.sort()
                _, _, p = cand[0]
                if c2d:
                    freeE[p] -= 1
                elif freeO[p] > 0:
                    freeO[p] -= 1
                else:
                    freeE[p] -= 1
                smax[p] = max(smax[p], L)
                placed.append((L, r, st, c2d, p))
            if not ok:
                continue
            tot_s = sum(smax)
            if best is None or tot_s < best[0]:
                best = (tot_s, placed)
        assert best is not None, "cell packing failed; raise npos"
        _, placed = best
        pieces = {r: [] for r in seqs}      # r -> [(g,p,start,len)]
        nextE = {p: 0 for p in range(NPOS)}   # even groups 0,2,4,6
        nextO = {p: 1 for p in range(NPOS)}   # odd groups 1,3,5,7
        usedE = {p: [] for p in range(NPOS)}
        for (L, r, st, c2d, p) in placed:
            if c2d:
                g_ = nextE[p]
                nextE[p] += 2
            else:
                if nextO[p] <= 7:
                    g_ = nextO[p]
                    nextO[p] += 2
                else:
                    g_ = nextE[p]
                    nextE[p] += 2
            assert g_ <= 7, "cell overflow"
            if c2d:
                pieces[r].insert(0, (g_, p, st, L))
            else:
                pieces[r].append((g_, p, st, L))
        self.pieces = pieces

        # per (g,p): the piece's windows (at most one piece per cell)
        cellw = [[[] for _ in range(NPOS)] for _ in range(G)]
        for r, pl in pieces.items():
            for (g_, p, st, ln) in pl:
                assert not cellw[g_][p], "cell already occupied"
                cellw[g_][p] = list(seqs[r][st:st + ln])

        # ---- slot list ----
        # chains: 'au' + 'pos0..NPOS-1'; slot = (chain, [win per group])
        self.chain_names = ['au'] + [f'pos{p}' for p in range(NPOS)]
        chain_slots = {'au': []}
        for j in range(self.S_au):
            wins = []
            for g_ in range(G):
                t = subs[g_][1]
                wins.append(t[j] if j < len(t) else None)
            chain_slots['au'].append(wins)
        for p in range(NPOS):
            sl = []
            mx = max(len(cellw[g_][p]) for g_ in range(G))
            for j in range(mx):
                sl.append([cellw[g_][p][j] if j < len(cellw[g_][p]) else None
                          for g_ in range(G)])
            chain_slots[f'pos{p}'] = sl
        self.chain_slots = chain_slots

        # ---- emission order ----
        # Phase 1: the entire au chain as singles (its DRAM bounce gates the
        # pa prefix, which gates C2 -- the kernel's critical path), eating the
        # ~95ns per-step RAW stall.  Phase 2: pos-chain first-copies.  Phase
        # 3: rotate pos-chain pairs so consecutive instructions share no
        # chain.
        ptr = {c: 0 for c in self.chain_names}
        nleft = {c: len(chain_slots[c]) for c in self.chain_names}
        emit = []        # (chain1, i1, chain2|None, i2)

        def take(c):
            i = ptr[c]
            ptr[c] += 1
            nleft[c] -= 1
            return i

        while nleft['au'] > 0:
            emit.append(('au', take('au'), None, 0))
        posn = [c for c in self.chain_names if c != 'au']
        for i in range(0, len(posn) - 1, 2):
            emit.append((posn[i], take(posn[i]), posn[i + 1],
                         take(posn[i + 1])))
        if len(posn) % 2:
            emit.append((posn[-1], take(posn[-1]), None, 0))
        prev = set()
        while any(nleft[c] > 0 for c in posn):
            cands = [c for c in posn if nleft[c] > 0]
            fresh = [c for c in cands if c not in prev]
            pool = fresh if fresh else cands
            pool.sort(key=lambda c: -nleft[c])
            c1 = pool[0]
            c2 = pool[1] if len(pool) > 1 else None
            if c2 is None:
                for c in cands:
                    if c != c1:
                        c2 = c
                        break
            i1 = take(c1)
            if c2 is None:
                emit.append((c1, i1, None, 0))
                prev = {c1}
            else:
                emit.append((c1, i1, c2, take(c2)))
                prev = {c1, c2}
        self.emit = emit

        # assign uw slot index in emission order
        slotmap = {}
        nxt = [0]
        for (c1, i1, c2, i2) in emit:
            slotmap[(c1, i1)] = nxt[0]
            nxt[0] += 1
            if c2 is not None:
                slotmap[(c2, i2)] = nxt[0]
                nxt[0] += 1
        self.S_total = nxt[0]
        self.slotmap = slotmap
        # au region end (for DMA chunk 0): last au slot index + 1
        self.au_end = max(slotmap[('au', j)] for j in range(self.S_au)) + 1

        # ---- C2 narrow steps (bc >= 1) on the row's first piece cell ----
        self.c2n = []
        for r in range(PB):
            g_, p, _, _ = pieces[r][0]
            for (bc2, s2) in gC2[r]:
                if bc2 >= 1:
                    self.c2n.append((g_, p, bc2, s2))

        # ---- host index table IDX8 [G, S_total, Z] into u_ext [16, K+1] ----
        zcol = K
        idx = np.full((G, self.S_total, Z), zcol, np.int32)
        zz = np.arange(Z)
        for (c, slots) in chain_slots.items():
            for j, wins in enumerate(slots):
                t = slotmap[(c, j)]
                for g_, w in enumerate(wins):
                    if w is None:
                        continue
                    bc, s = w
                    cols = bc * Z + (zz + s) % Z
                    if bc == 20:
                        cols = np.where((zz + s) % Z < 320, cols, zcol)
                    elif bc >= 21:
                        cols = np.full(Z, zcol)
                    idx[g_, t] = cols
        self.IDX8 = idx


def _build_program(plan):
    import concourse.tile as tile
    from concourse import bacc, mybir
    from concourse.alu_op_type import AluOpType
    import bass_rust

    u16 = mybir.dt.uint16
    XOR = AluOpType.bitwise_xor
    VecI64Pair = bass_rust.VecI64Pair
    NPOS = plan.NPOS

    nc = bacc.Bacc("TRN2", target_bir_lowering=False, debug=False)
    S = plan.S_total
    uw_d = nc.dram_tensor("uw", [128, S * Z], u16, kind="ExternalInput").ap()
    opb_d = nc.dram_tensor("opb", [128, NPOS * Z], u16,
                           kind="ExternalOutput").ap()
    opa_d = nc.dram_tensor("opa", [16, 4 * Z], u16, kind="ExternalOutput").ap()

    def pair_view(flat_ap, addr_a, addr_b, ln=Z):
        v = flat_ap[:, addr_a:addr_a + 1]
        w = v.copy()
        pstride = v.ap.to_list()[0]
        w.ap = VecI64Pair([pstride, [addr_b - addr_a, 2], [1, ln]])
        return w

    with tile.TileContext(nc) as tc, ExitStack() as ctx:
        pin = ctx.enter_context(tc.tile_pool(name="pin", bufs=1))
        pw = ctx.enter_context(tc.tile_pool(name="pw", bufs=1))
        pdram = ctx.enter_context(tc.tile_pool(name="pdram", bufs=1,
                                               space="DRAM"))

        uw = pin.tile([128, S * Z], u16, tag="uw")
        acc = pw.tile([128, (1 + NPOS) * Z], u16, tag="acc")  # au | pos cells
        aus = pw.tile([128, G * Z], u16, tag="aus")   # regrouped+replicated
        scr = pw.tile([128, 2 * Z], u16, tag="scr")   # merge scratch
        pa = pw.tile([128, 4 * 2 * Z], u16, tag="pa")  # halo'd pa, all groups
        bau = pdram.tile([16, G * Z], u16, tag="bau")

        # ---- input DMA, chunked along slots (tiny first chunk, then grow;
        # a cut right at the au-region end so au lands early) ----
        ae = plan.au_end
        cuts = sorted(set(min(c, S) for c in
                          [0, 3, ae, ae + (S - ae) // 3,
                           ae + 2 * (S - ae) // 3, S]))
        for a, b in zip(cuts[:-1], cuts[1:]):
            if b > a:
                nc.sync.dma_start(uw[:, a * Z:b * Z], uw_d[:, a * Z:b * Z])

        # ---- slot instructions ----
        dsta = {'au': 0}
        for p in range(NPOS):
            dsta[f'pos{p}'] = (1 + p) * Z

        def emit_slot(c1, i1, c2, i2):
            first = (i1 == 0)
            d1 = dsta[c1]
            s1 = plan.slotmap[(c1, i1)] * Z
            if c2 is None:
                if first:
                    nc.vector.tensor_copy(acc[:, d1:d1 + Z], uw[:, s1:s1 + Z])
                else:
                    nc.vector.tensor_tensor(acc[:, d1:d1 + Z],
                                            acc[:, d1:d1 + Z],
                                            uw[:, s1:s1 + Z], op=XOR)
                return
            d2 = dsta[c2]
            s2 = plan.slotmap[(c2, i2)] * Z
            dst = pair_view(acc, d1, d2)
            src = pair_view(uw, s1, s2)
            if first:
                nc.vector.tensor_copy(dst, src)
            else:
                nc.vector.tensor_tensor(dst, pair_view(acc, d1, d2), src,
                                        op=XOR)

        # emit until au chain is complete, then the bounce DMAs; merges are
        # deferred TUNE["defer"] instructions further so the DVE queue does
        # not head-of-line stall on the bounce latency
        au_done_at = 0
        for k, (c1, i1, c2, i2) in enumerate(plan.emit):
            if (c1 == 'au' and i1 == plan.S_au - 1) or \
               (c2 == 'au' and i2 == plan.S_au - 1):
                au_done_at = k
        merge_at = min(au_done_at + TUNE["defer"], len(plan.emit) - 1)
        for k, (c1, i1, c2, i2) in enumerate(plan.emit):
            emit_slot(c1, i1, c2, i2)
            if k == au_done_at:
                # ---- au bounce: transposed write, regroup+replicate read ----
                # write: bau[l*G*Z + c*Z + z] = acc_au[16c+l, z]
                dst = bau[:, :]
                dv = dst.copy()
                dv.ap = VecI64Pair([[Z, G], [G * Z, 16], [1, Z]])
                nc.sync.dma_start(dv, acc[:, 0:Z])
                # read: aus[16d+l, c*Z+z] = bau[l*G*Z + c*Z + z] (dup over d)
                src = bau[:, :]
                sv = src.copy()
                sv.ap = VecI64Pair([[0, G], [G * Z, 16], [1, G * Z]])
                nc.sync.dma_start(aus, sv)
            if k == merge_at:
                # ---- merge sub-chains into row values ----
                subrows = {}
                for c_, (r, terms) in enumerate(plan.au_subs):
                    if terms:
                        subrows.setdefault(r, []).append(c_ * Z)
                rowaddr = {}
                perrow = {}          # r -> [(dst, in0, in1)]
                scrn = 0
                for r in range(4):
                    lst = subrows.get(r, [])
                    assert lst, "au row with no sub-chain"
                    if len(lst) == 1:
                        rowaddr[r] = ('aus', lst[0])
                    else:
                        cur = ('aus', lst[0])
                        ops = []
                        da = scrn * Z
                        for x in lst[1:]:
                            ops.append((da, cur, ('aus', x)))
                            cur = ('scr', da)
                        scrn = (scrn + 1) % 2
                        perrow[r] = ops
                        rowaddr[r] = cur
                # round-robin interleave rows' merge chains so adjacent ops
                # come from different rows (pairable without RAW hazards)
                merge_ops = []
                mk = 0
                while any(perrow.values()):
                    keys = [r for r in perrow if perrow[r]]
                    r = keys[mk % len(keys)]
                    mk += 1
                    merge_ops.append(perrow[r].pop(0))
                tiles = {'aus': aus, 'scr': scr}
                i = 0
                while i < len(merge_ops):
                    if i + 1 < len(merge_ops):
                        (da1, a1, b1), (da2, a2, b2) = merge_ops[i], \
                            merge_ops[i + 1]
                        if a1[0] == a2[0] and b1[0] == b2[0] and da1 != da2:
                            nc.vector.tensor_tensor(
                                pair_view(scr, da1, da2),
                                pair_view(tiles[a1[0]], a1[1], a2[1]),
                                pair_view(tiles[b1[0]], b1[1], b2[1]), op=XOR)
                            i += 2
                            continue
                    (da1, a1, b1) = merge_ops[i]
                    nc.vector.tensor_tensor(scr[:, da1:da1 + Z],
                                            tiles[a1[0]][:, a1[1]:a1[1] + Z],
                                            tiles[b1[0]][:, b1[1]:b1[1] + Z],
                                            op=XOR)
                    i += 1

                # ---- prefix into halo'd pa (full width, all groups) ----
                t0, a0 = rowaddr[0]
                nc.vector.tensor_copy(pair_view(pa, 0, Z),
                                      pair_view(tiles[t0], a0, a0))
                for r in range(1, 4):
                    tr, ar = rowaddr[r]
                    nc.vector.tensor_tensor(
                        pair_view(pa, r * 2 * Z, r * 2 * Z + Z),
                        pair_view(pa, (r - 1) * 2 * Z, (r - 1) * 2 * Z),
                        pair_view(tiles[tr], ar, ar), op=XOR)

                # pa output (main halves, lanes = partitions 0..15)
                nc.sync.dma_start(
                    opa_d.rearrange("p (b z) -> p b z", z=Z),
                    pa.rearrange("p (b z) -> p b z", z=2 * Z)[0:16, :, 0:Z])

        # ---- C2 narrow XOR (bc>=1) on even-group cells.  Pair two steps of
        # the same group when they hit DIFFERENT cells (pair_view works on
        # 32-aligned partition bases).  Order by bc so early prefix blocks
        # unblock C2 sooner; rotate groups to avoid per-cell RAW stalls.
        # Emit each position's output DMA right after its last C2 op. ----
        bygroup = {}
        for (g_, p_, bc, s) in plan.c2n:
            bygroup.setdefault(g_, []).append((p_, bc, s))
        gops = {}
        for g_, items in bygroup.items():
            items.sort(key=lambda x: x[1])
            ops = []
            used = [False] * len(items)
            for i in range(len(items)):
                if used[i]:
                    continue
                used[i] = True
                part = None
                for j in range(i + 1, len(items)):
                    if not used[j] and items[j][0] != items[i][0]:
                        part = j
                        break
                if part is None:
                    ops.append((items[i],))
                else:
                    used[part] = True
                    ops.append((items[i], items[part]))
            gops[g_] = ops
        pending = {p: 0 for p in range(NPOS)}
        for g_, ops in gops.items():
            for op in ops:
                for (p_, _, _) in op:
                    pending[p_] += 1
        for p_ in range(NPOS):
            if pending[p_] == 0:
                nc.sync.dma_start(opb_d[:, p_ * Z:(p_ + 1) * Z],
                                  acc[:, (1 + p_) * Z:(2 + p_) * Z])
        queues = [gops[g_] for g_ in sorted(gops)]
        gids = sorted(gops)
        k = 0
        while any(queues):
            qi = k % len(queues)
            k += 1
            if not queues[qi]:
                continue
            op = queues[qi].pop(0)
            g_ = gids[qi]
            sub = slice(g_ * PL, (g_ + 1) * PL)
            if len(op) == 1:
                (p1, b1, s1) = op[0]
                d1 = (1 + p1) * Z
                a1 = b1 * 2 * Z + s1
                nc.vector.tensor_tensor(
                    acc[sub, d1:d1 + Z], acc[sub, d1:d1 + Z],
                    pa[sub, a1:a1 + Z], op=XOR)
            else:
                (p1, b1, s1), (p2, b2, s2) = op
                d1, d2 = (1 + p1) * Z, (1 + p2) * Z
                a1 = b1 * 2 * Z + s1
                a2 = b2 * 2 * Z + s2
                accg = acc[sub, :]
                pag = pa[sub, :]
                nc.vector.tensor_tensor(pair_view(accg, d1, d2),
                                        pair_view(accg, d1, d2),
                                        pair_view(pag, a1, a2), op=XOR)
            for (p_, _, _) in op:
                pending[p_] -= 1
                if pending[p_] == 0:
                    nc.sync.dma_start(opb_d[:, p_ * Z:(p_ + 1) * Z],
                                      acc[:, (1 + p_) * Z:(2 + p_) * Z])

    return nc


def _get_plan_program(a_rows, a_cols, bi_rows, bi_cols, c1_rows, c1_cols,
                      c2_rows, c2_cols):
    if "prog" in _CACHE:
        return _CACHE["plan"], _CACHE["prog"]
    entB = _base_entries(bi_rows, bi_cols)
    assert sorted(entB) == [(i, j, 0) for i in range(4) for j in range(i + 1)]
    gA = _group(_base_entries(a_rows, a_cols), 4, drop_bc=(21,))
    gC1 = _group(_base_entries(c1_rows, c1_cols), PB, drop_bc=(21,))
    gC2 = _group(_base_entries(c2_rows, c2_cols), PB)
    plan = Plan(gA, gC1, gC2)
    nc = _build_program(plan)
    nc.compile()
    _CACHE["plan"] = plan
    _CACHE["prog"] = nc
    return plan, nc


def kernel(u, a_rows, a_cols, bi_rows, bi_cols, c1_rows, c1_cols,
           c2_rows, c2_cols, out_int, **_ignored):
    from concourse.bass_utils import run_bass_kernel_spmd

    u = np.asarray(u)
    assert u.shape == (B_TOTAL, K)
    plan, nc = _get_plan_program(a_rows, a_cols, bi_rows, bi_cols,
                                 c1_rows, c1_cols, c2_rows, c2_cols)

    # ---- host marshalling: pack 16 batch rows per uint16 lane ----
    ub = u.astype(np.uint16)
    p128 = np.arange(128)
    lane = p128 % PL
    grp = p128 // PL
    in_maps = []
    for c in range(N_CORES):
        seg = ub[c * B_CORE:(c + 1) * B_CORE]          # [256, 8000]
        packed = np.zeros((PL, K), np.uint16)
        for t in range(PACK):
            packed |= (seg[t * PL:(t + 1) * PL] << t).astype(np.uint16)
        u_ext = np.concatenate([packed, np.zeros((PL, 1), np.uint16)], axis=1)
        uwc = u_ext[lane[:, None, None], plan.IDX8[grp]]   # [128, S, 384]
        in_maps.append({"uw": np.ascontiguousarray(
            uwc.reshape(128, plan.S_total * Z))})

    res = run_bass_kernel_spmd(nc, in_maps, core_ids=list(range(N_CORES)))

    # ---- host assembly ----
    oi = np.asarray(out_int)
    out = np.empty((B_TOTAL, N), np.float32)
    shift = np.arange(PACK, dtype=np.uint16)
    for c in range(N_CORES):
        opa = np.asarray(res.results[c]["opa"])        # [16, 1536]
        opb = np.asarray(res.results[c]["opb"])        # [128, NPOS*384]
        cs = np.empty((B_CORE, N), np.float32)
        cs[:, 0:K - 2 * Z] = u[c * B_CORE:(c + 1) * B_CORE, 2 * Z:K]
        pa_bits = ((opa[None, :, :] >> shift[:, None, None]) & 1)
        cs[:, K - 2 * Z:K - 2 * Z + PA_BITS] = (
            pa_bits.reshape(B_CORE, PA_BITS))
        pb = np.empty((B_CORE, PB * Z), np.float32)
        for r in range(PB):
            w = np.zeros((PL, Z), np.uint16)
            for (g_, p_, _, _) in plan.pieces[r]:
                w ^= opb[g_ * PL:(g_ + 1) * PL, p_ * Z:(p_ + 1) * Z]
            bits = ((w[None, :, :] >> shift[:, None, None]) & 1)
            pb[:, r * Z:(r + 1) * Z] = bits.reshape(B_CORE, Z)
        cs[:, K - 2 * Z + PA_BITS:] = pb[:, :PB_BITS]
        out[c * B_CORE:(c + 1) * B_CORE] = cs[:, oi]
    return out


# revision 19
# speedup vs baseline: 1.0541x; 1.0394x over previous
"""5G LDPC BG1 encoder (k=8000, n=16000, r=0.5, Z=384) on 8 Trainium2 cores.

Strategy (v5): batch data-parallelism (2048 -> 8 cores x 256 codewords) with
16-way bit packing: 16 codewords share one uint16 SBUF lane (bit t of lane l =
codeword t*16+l), so a core's batch fits in 16 partitions.  The 128 partitions
form 8 groups x 16 lanes; every group computes DIFFERENT parity rows over the
SAME free-dim offsets, so one [128, 2, 384] DVE bitwise-XOR advances 16 GF(2)
chain steps at once.  This relies on the host pre-aligning every circulant-
shifted operand window into a slot-stream input uw[128, S, 384] (pure gather/
layout marshalling, the same class of work as the bit packing itself).  The
core parity pa = B^-1(A u) is built from 8 group-parallel au sub-chains that
are merged across partition groups via a two-hop DRAM bounce whose transposed
write makes the read-back a single regroup+replicate DMA into all 8 groups;
the prefix then runs full-width so every group owns a halo'd pa replica.  C2
terms on pa block 0 (= A-row-0 sum) are expanded into u-windows and ride the
slot stream; remaining C2 terms run as narrow per-group XOR pairs.  Long rows
are split into at most two slot cells whose parities the host XORs during
unpack.  Only parity bits leave the chip; the host assembles the final
codeword from its own u plus device parity, applying the static rate-matching
interleaver while unpacking.
"""
import numpy as np
from contextlib import ExitStack

Z = 384
KB = 22
K = 8000
N = 16000
K_LDPC = KB * Z          # 8448
PB = 19                  # pb blocks that survive rate matching
PB_BITS = 7232           # pb bits used (18*384 + 320)
PA_BITS = 4 * Z          # 1536

B_TOTAL = 2048
N_CORES = 8
B_CORE = B_TOTAL // N_CORES   # 256
PACK = 16                     # codewords per uint16 lane
PL = 16                       # partitions (lanes) per group
G = 8                         # partition groups

_CACHE = {}

TUNE = {
    "npos": 4,            # pb accumulator cells per group
    "defer": 11,          # slot insts between au bounce and merge emission
}


def _base_entries(rows, cols):
    rows = np.asarray(rows, np.int64)
    cols = np.asarray(cols, np.int64)
    m = (rows % Z) == 0
    br = (rows[m] // Z).astype(int)
    bc = (cols[m] // Z).astype(int)
    sh = (cols[m] % Z).astype(int)
    return list(zip(br.tolist(), bc.tolist(), sh.tolist()))


def _group(entries, n_blocks, drop_bc=()):
    g = [[] for _ in range(n_blocks)]
    for br, bc, s in entries:
        if bc in drop_bc or br >= n_blocks:
            continue
        g[br].append((bc, s))
    return g


class Plan:
    """Static schedule: slot stream, cell map, narrow step lists, host maps."""

    def __init__(self, gA, gC1, gC2):
        self.gA, self.gC1, self.gC2 = gA, gC1, gC2
        NPOS = TUNE["npos"]
        self.NPOS = NPOS

        # ---- au sub-chains: split the A rows into G chains ----
        total = sum(len(g) for g in gA)
        tgt = max(1, -(-total // G))
        subs = []                      # (row, [terms])
        for r in range(4):
            t = list(gA[r])
            np_ = min(max(1, -(-len(t) // tgt)), len(t))
            sizes = [len(t) // np_ + (1 if i < len(t) % np_ else 0)
                     for i in range(np_)]
            o = 0
            for sz in sizes:
                subs.append((r, t[o:o + sz]))
                o += sz
        while len(subs) > G:
            subs.sort(key=lambda x: len(x[1]))
            a = subs.pop(0)
            for i, b in enumerate(subs):
                if b[0] == a[0]:
                    subs[i] = (b[0], b[1] + a[1])
                    break
            else:
                subs.append(a)
                break
        while len(subs) < G:
            subs.append((0, []))       # empty pad chain (zero windows)
        subs.sort(key=lambda x: -len(x[1]))
        self.au_subs = subs
        self.S_au = max(len(t) for _, t in subs)

        # ---- pb row sequences: C1 terms + expanded bc0 C2 windows ----
        exp0 = gA[0]
        seqs = {}
        for r in range(PB):
            sq = list(gC1[r])
            for (bc2, s2) in gC2[r]:
                if bc2 == 0:
                    sq += [(bc1, (s1 + s2) % Z) for (bc1, s1) in exp0]
            seqs[r] = sq

        # ---- pack row pieces into G x NPOS cells (ONE piece per cell) ----
        # Rows split into <=2 pieces; total slots = sum_p max piece len at p.
        # Constraint: the piece receiving a row's C2 (bc>=1) narrow adds must
        # sit on an EVEN group: engine ops on partition-sliced APs only
        # compile when the partition base is a multiple of 32.
        lens = {r: len(s) for r, s in seqs.items()}
        has_c2 = {r: any(bc >= 1 for (bc, _) in gC2[r]) for r in range(PB)}
        maxlen = max(lens.values())
        best = None
        for c0 in range(3, maxlen + 1):
            pcs = []                        # (length, row, start, is_c2dst)
            ok = True
            for r, L in lens.items():
                if L > c0:
                    # the SMALL piece carries the row's C2 adds -- big pieces
                    # then pack freely instead of competing for even cells
                    pcs.append((c0, r, 0, False))
                    pcs.append((L - c0, r, c0, has_c2[r]))
                else:
                    pcs.append((L, r, 0, has_c2[r]))
            if len(pcs) > G * NPOS:
                continue
            pcs.sort(key=lambda x: (-x[0], -x[3]))
            # greedy place: per position 4 even + 4 odd cells
            smax = [0] * NPOS
            freeE = [4] * NPOS
            freeO = [4] * NPOS
            placed = []
            for (L, r, st, c2d) in pcs:
                cand = []
                for p in range(NPOS):
                    if c2d and freeE[p] == 0:
                        continue
                    if not c2d and freeE[p] + freeO[p] == 0:
                        continue
                    grow = max(0, L - smax[p])
                    cand.append((grow, -smax[p], p))
                if not cand:
                    ok = False
                    break
                cand.sort()
                _, _, p = cand[0]
                if c2d:
                    freeE[p] -= 1
                elif freeO[p] > 0:
                    freeO[p] -= 1
                else:
                    freeE[p] -= 1
                smax[p] = max(smax[p], L)
                placed.append((L, r, st, c2d, p))
            if not ok:
                continue
            tot_s = sum(smax)
            if best is None or tot_s < best[0]:
                best = (tot_s, placed)
        assert best is not None, "cell packing failed; raise npos"
        _, placed = best
        pieces = {r: [] for r in seqs}      # r -> [(g,p,start,len)]
        nextE = {p: 0 for p in range(NPOS)}   # even groups 0,2,4,6
        nextO = {p: 1 for p in range(NPOS)}   # odd groups 1,3,5,7
        usedE = {p: [] for p in range(NPOS)}
        for (L, r, st, c2d, p) in placed:
            if c2d:
                g_ = nextE[p]
                nextE[p] += 2
            else:
                if nextO[p] <= 7:
                    g_ = nextO[p]
                    nextO[p] += 2
                else:
                    g_ = nextE[p]
                    nextE[p] += 2
            assert g_ <= 7, "cell overflow"
            if c2d:
                pieces[r].insert(0, (g_, p, st, L))
            else:
                pieces[r].append((g_, p, st, L))
        self.pieces = pieces

        # per (g,p): the piece's windows (at most one piece per cell)
        cellw = [[[] for _ in range(NPOS)] for _ in range(G)]
        for r, pl in pieces.items():
            for (g_, p, st, ln) in pl:
                assert not cellw[g_][p], "cell already occupied"
                cellw[g_][p] = list(seqs[r][st:st + ln])

        # ---- slot list ----
        # chains: 'au' + 'pos0..NPOS-1'; slot = (chain, [win per group])
        self.chain_names = ['au'] + [f'pos{p}' for p in range(NPOS)]
        chain_slots = {'au': []}
        for j in range(self.S_au):
            wins = []
            for g_ in range(G):
                t = subs[g_][1]
                wins.append(t[j] if j < len(t) else None)
            chain_slots['au'].append(wins)
        for p in range(NPOS):
            sl = []
            mx = max(len(cellw[g_][p]) for g_ in range(G))
            for j in range(mx):
                sl.append([cellw[g_][p][j] if j < len(cellw[g_][p]) else None
                          for g_ in range(G)])
            chain_slots[f'pos{p}'] = sl
        self.chain_slots = chain_slots

        # ---- emission order ----
        # Phase 1: the entire au chain as singles (its DRAM bounce gates the
        # pa prefix, which gates C2 -- the kernel's critical path), eating the
        # ~95ns per-step RAW stall.  Phase 2: pos-chain first-copies.  Phase
        # 3: rotate pos-chain pairs so consecutive instructions share no
        # chain.
        ptr = {c: 0 for c in self.chain_names}
        nleft = {c: len(chain_slots[c]) for c in self.chain_names}
        emit = []        # (chain1, i1, chain2|None, i2)

        def take(c):
            i = ptr[c]
            ptr[c] += 1
            nleft[c] -= 1
            return i

        while nleft['au'] > 0:
            emit.append(('au', take('au'), None, 0))
        posn = [c for c in self.chain_names if c != 'au']
        for i in range(0, len(posn) - 1, 2):
            emit.append((posn[i], take(posn[i]), posn[i + 1],
                         take(posn[i + 1])))
        if len(posn) % 2:
            emit.append((posn[-1], take(posn[-1]), None, 0))
        prev = set()
        while any(nleft[c] > 0 for c in posn):
            cands = [c for c in posn if nleft[c] > 0]
            fresh = [c for c in cands if c not in prev]
            pool = fresh if fresh else cands
            pool.sort(key=lambda c: -nleft[c])
            c1 = pool[0]
            c2 = pool[1] if len(pool) > 1 else None
            if c2 is None:
                for c in cands:
                    if c != c1:
                        c2 = c
                        break
            i1 = take(c1)
            if c2 is None:
                emit.append((c1, i1, None, 0))
                prev = {c1}
            else:
                emit.append((c1, i1, c2, take(c2)))
                prev = {c1, c2}
        self.emit = emit

        # assign uw slot index in emission order
        slotmap = {}
        nxt = [0]
        for (c1, i1, c2, i2) in emit:
            slotmap[(c1, i1)] = nxt[0]
            nxt[0] += 1
            if c2 is not None:
                slotmap[(c2, i2)] = nxt[0]
                nxt[0] += 1
        self.S_total = nxt[0]
        self.slotmap = slotmap
        # au region end (for DMA chunk 0): last au slot index + 1
        self.au_end = max(slotmap[('au', j)] for j in range(self.S_au)) + 1

        # ---- C2 narrow steps (bc >= 1) on the row's first piece cell ----
        self.c2n = []
        for r in range(PB):
            g_, p, _, _ = pieces[r][0]
            for (bc2, s2) in gC2[r]:
                if bc2 >= 1:
                    self.c2n.append((g_, p, bc2, s2))

        # ---- host index table IDX8 [G, S_total, Z] into u_ext [16, K+1] ----
        zcol = K
        idx = np.full((G, self.S_total, Z), zcol, np.int32)
        zz = np.arange(Z)
        for (c, slots) in chain_slots.items():
            for j, wins in enumerate(slots):
                t = slotmap[(c, j)]
                for g_, w in enumerate(wins):
                    if w is None:
                        continue
                    bc, s = w
                    cols = bc * Z + (zz + s) % Z
                    if bc == 20:
                        cols = np.where((zz + s) % Z < 320, cols, zcol)
                    elif bc >= 21:
                        cols = np.full(Z, zcol)
                    idx[g_, t] = cols
        self.IDX8 = idx


def _build_program(plan):
    import concourse.tile as tile
    from concourse import bacc, mybir
    from concourse.alu_op_type import AluOpType
    import bass_rust

    u16 = mybir.dt.uint16
    XOR = AluOpType.bitwise_xor
    VecI64Pair = bass_rust.VecI64Pair
    NPOS = plan.NPOS

    nc = bacc.Bacc("TRN2", target_bir_lowering=False, debug=False)
    S = plan.S_total
    uw_d = nc.dram_tensor("uw", [128, S * Z], u16, kind="ExternalInput").ap()
    opb_d = nc.dram_tensor("opb", [128, NPOS * Z], u16,
                           kind="ExternalOutput").ap()
    opa_d = nc.dram_tensor("opa", [16, 4 * Z], u16, kind="ExternalOutput").ap()

    def pair_view(flat_ap, addr_a, addr_b, ln=Z):
        v = flat_ap[:, addr_a:addr_a + 1]
        w = v.copy()
        pstride = v.ap.to_list()[0]
        w.ap = VecI64Pair([pstride, [addr_b - addr_a, 2], [1, ln]])
        return w

    with tile.TileContext(nc) as tc, ExitStack() as ctx:
        pin = ctx.enter_context(tc.tile_pool(name="pin", bufs=1))
        pw = ctx.enter_context(tc.tile_pool(name="pw", bufs=1))
        pdram = ctx.enter_context(tc.tile_pool(name="pdram", bufs=1,
                                               space="DRAM"))

        uw = pin.tile([128, S * Z], u16, tag="uw")
        acc = pw.tile([128, (1 + NPOS) * Z], u16, tag="acc")  # au | pos cells
        aus = pw.tile([128, G * Z], u16, tag="aus")   # regrouped+replicated
        scr = pw.tile([128, 2 * Z], u16, tag="scr")   # merge scratch
        pa = pw.tile([128, 4 * 2 * Z], u16, tag="pa")  # halo'd pa, all groups
        bau = pdram.tile([16, G * Z], u16, tag="bau")

        # ---- input DMA, chunked along slots (tiny first chunk, then grow;
        # a cut right at the au-region end so au lands early).  Only the
        # chunks up to `pre_bounce` are issued here; the rest are issued
        # after the au-bounce DMAs so the bounce isn't queued behind them
        # on the shared HWDGE/DMA path. ----
        ae = plan.au_end
        cuts = sorted(set(min(c, S) for c in
                          [0, 3, ae, ae + (S - ae) // 4,
                           ae + (S - ae) // 2, ae + 3 * (S - ae) // 4, S]))
        chunks = [(a, b) for a, b in zip(cuts[:-1], cuts[1:]) if b > a]
        pre_bounce = 3
        for a, b in chunks[:pre_bounce]:
            nc.sync.dma_start(uw[:, a * Z:b * Z], uw_d[:, a * Z:b * Z])

        # ---- slot instructions ----
        dsta = {'au': 0}
        for p in range(NPOS):
            dsta[f'pos{p}'] = (1 + p) * Z

        def emit_slot(c1, i1, c2, i2):
            first = (i1 == 0)
            d1 = dsta[c1]
            s1 = plan.slotmap[(c1, i1)] * Z
            if c2 is None:
                if first:
                    nc.vector.tensor_copy(acc[:, d1:d1 + Z], uw[:, s1:s1 + Z])
                else:
                    nc.vector.tensor_tensor(acc[:, d1:d1 + Z],
                                            acc[:, d1:d1 + Z],
                                            uw[:, s1:s1 + Z], op=XOR)
                return
            d2 = dsta[c2]
            s2 = plan.slotmap[(c2, i2)] * Z
            dst = pair_view(acc, d1, d2)
            src = pair_view(uw, s1, s2)
            if first:
                nc.vector.tensor_copy(dst, src)
            else:
                nc.vector.tensor_tensor(dst, pair_view(acc, d1, d2), src,
                                        op=XOR)

        # emit until au chain is complete, then the bounce DMAs; merges are
        # deferred TUNE["defer"] instructions further so the DVE queue does
        # not head-of-line stall on the bounce latency
        au_done_at = 0
        for k, (c1, i1, c2, i2) in enumerate(plan.emit):
            if (c1 == 'au' and i1 == plan.S_au - 1) or \
               (c2 == 'au' and i2 == plan.S_au - 1):
                au_done_at = k
        merge_at = min(au_done_at + TUNE["defer"], len(plan.emit) - 1)
        for k, (c1, i1, c2, i2) in enumerate(plan.emit):
            emit_slot(c1, i1, c2, i2)
            if k == au_done_at:
                # ---- au bounce: transposed write, regroup+replicate read ----
                # write: bau[l*G*Z + c*Z + z] = acc_au[16c+l, z]
                dst = bau[:, :]
                dv = dst.copy()
                dv.ap = VecI64Pair([[Z, G], [G * Z, 16], [1, Z]])
                nc.sync.dma_start(dv, acc[:, 0:Z])
                # read: aus[16d+l, c*Z+z] = bau[l*G*Z + c*Z + z], but only
                # into the even groups (C2 cells live there; odd-group pa is
                # never read) -- 4 small DMAs at 32-aligned partition bases
                src = bau[:, :]
                sv = src.copy()
                sv.ap = VecI64Pair([[G * Z, 16], [1, G * Z]])
                for dge in range(4):
                    nc.sync.dma_start(aus[32 * dge:32 * dge + 16, :], sv)
                # remaining input chunks go after the bounce in queue order
                for a, b in chunks[pre_bounce:]:
                    nc.sync.dma_start(uw[:, a * Z:b * Z],
                                      uw_d[:, a * Z:b * Z])
            if k == merge_at:
                # ---- merge sub-chains into row values ----
                subrows = {}
                for c_, (r, terms) in enumerate(plan.au_subs):
                    if terms:
                        subrows.setdefault(r, []).append(c_ * Z)
                rowaddr = {}
                perrow = {}          # r -> [(dst, in0, in1)]
                scrn = 0
                for r in range(4):
                    lst = subrows.get(r, [])
                    assert lst, "au row with no sub-chain"
                    if len(lst) == 1:
                        rowaddr[r] = ('aus', lst[0])
                    else:
                        cur = ('aus', lst[0])
                        ops = []
                        da = scrn * Z
                        for x in lst[1:]:
                            ops.append((da, cur, ('aus', x)))
                            cur = ('scr', da)
                        scrn = (scrn + 1) % 2
                        perrow[r] = ops
                        rowaddr[r] = cur
                # round-robin interleave rows' merge chains so adjacent ops
                # come from different rows (pairable without RAW hazards)
                merge_ops = []
                mk = 0
                while any(perrow.values()):
                    keys = [r for r in perrow if perrow[r]]
                    r = keys[mk % len(keys)]
                    mk += 1
                    merge_ops.append(perrow[r].pop(0))
                tiles = {'aus': aus, 'scr': scr}
                i = 0
                while i < len(merge_ops):
                    if i + 1 < len(merge_ops):
                        (da1, a1, b1), (da2, a2, b2) = merge_ops[i], \
                            merge_ops[i + 1]
                        if a1[0] == a2[0] and b1[0] == b2[0] and da1 != da2:
                            nc.vector.tensor_tensor(
                                pair_view(scr, da1, da2),
                                pair_view(tiles[a1[0]], a1[1], a2[1]),
                                pair_view(tiles[b1[0]], b1[1], b2[1]), op=XOR)
                            i += 2
                            continue
                    (da1, a1, b1) = merge_ops[i]
                    nc.vector.tensor_tensor(scr[:, da1:da1 + Z],
                                            tiles[a1[0]][:, a1[1]:a1[1] + Z],
                                            tiles[b1[0]][:, b1[1]:b1[1] + Z],
                                            op=XOR)
                    i += 1

                # ---- prefix into halo'd pa (full width, all groups) ----
                t0, a0 = rowaddr[0]
                nc.vector.tensor_copy(pair_view(pa, 0, Z),
                                      pair_view(tiles[t0], a0, a0))
                for r in range(1, 4):
                    tr, ar = rowaddr[r]
                    nc.vector.tensor_tensor(
                        pair_view(pa, r * 2 * Z, r * 2 * Z + Z),
                        pair_view(pa, (r - 1) * 2 * Z, (r - 1) * 2 * Z),
                        pair_view(tiles[tr], ar, ar), op=XOR)

                # pa output (main halves, lanes = partitions 0..15)
                nc.sync.dma_start(
                    opa_d.rearrange("p (b z) -> p b z", z=Z),
                    pa.rearrange("p (b z) -> p b z", z=2 * Z)[0:16, :, 0:Z])

        # ---- C2 narrow XOR (bc>=1) on even-group cells.  Pair two steps of
        # the same group when they hit DIFFERENT cells (pair_view works on
        # 32-aligned partition bases).  Order by bc so early prefix blocks
        # unblock C2 sooner; rotate groups to avoid per-cell RAW stalls.
        # Emit each position's output DMA right after its last C2 op. ----
        bygroup = {}
        for (g_, p_, bc, s) in plan.c2n:
            bygroup.setdefault(g_, []).append((p_, bc, s))
        gops = {}
        for g_, items in bygroup.items():
            items.sort(key=lambda x: x[1])
            ops = []
            used = [False] * len(items)
            for i in range(len(items)):
                if used[i]:
                    continue
                used[i] = True
                part = None
                for j in range(i + 1, len(items)):
                    if not used[j] and items[j][0] != items[i][0]:
                        part = j
                        break
                if part is None:
                    ops.append((items[i],))
                else:
                    used[part] = True
                    ops.append((items[i], items[part]))
            gops[g_] = ops
        pending = {p: 0 for p in range(NPOS)}
        for g_, ops in gops.items():
            for op in ops:
                for (p_, _, _) in op:
                    pending[p_] += 1
        for p_ in range(NPOS):
            if pending[p_] == 0:
                nc.sync.dma_start(opb_d[:, p_ * Z:(p_ + 1) * Z],
                                  acc[:, (1 + p_) * Z:(2 + p_) * Z])
        queues = [gops[g_] for g_ in sorted(gops)]
        gids = sorted(gops)
        k = 0
        while any(queues):
            qi = k % len(queues)
            k += 1
            if not queues[qi]:
                continue
            op = queues[qi].pop(0)
            g_ = gids[qi]
            sub = slice(g_ * PL, (g_ + 1) * PL)
            if len(op) == 1:
                (p1, b1, s1) = op[0]
                d1 = (1 + p1) * Z
                a1 = b1 * 2 * Z + s1
                nc.vector.tensor_tensor(
                    acc[sub, d1:d1 + Z], acc[sub, d1:d1 + Z],
                    pa[sub, a1:a1 + Z], op=XOR)
            else:
                (p1, b1, s1), (p2, b2, s2) = op
                d1, d2 = (1 + p1) * Z, (1 + p2) * Z
                a1 = b1 * 2 * Z + s1
                a2 = b2 * 2 * Z + s2
                accg = acc[sub, :]
                pag = pa[sub, :]
                nc.vector.tensor_tensor(pair_view(accg, d1, d2),
                                        pair_view(accg, d1, d2),
                                        pair_view(pag, a1, a2), op=XOR)
            for (p_, _, _) in op:
                pending[p_] -= 1
                if pending[p_] == 0:
                    nc.sync.dma_start(opb_d[:, p_ * Z:(p_ + 1) * Z],
                                      acc[:, (1 + p_) * Z:(2 + p_) * Z])

    return nc


def _get_plan_program(a_rows, a_cols, bi_rows, bi_cols, c1_rows, c1_cols,
                      c2_rows, c2_cols):
    if "prog" in _CACHE:
        return _CACHE["plan"], _CACHE["prog"]
    entB = _base_entries(bi_rows, bi_cols)
    assert sorted(entB) == [(i, j, 0) for i in range(4) for j in range(i + 1)]
    gA = _group(_base_entries(a_rows, a_cols), 4, drop_bc=(21,))
    gC1 = _group(_base_entries(c1_rows, c1_cols), PB, drop_bc=(21,))
    gC2 = _group(_base_entries(c2_rows, c2_cols), PB)
    plan = Plan(gA, gC1, gC2)
    nc = _build_program(plan)
    nc.compile()
    _CACHE["plan"] = plan
    _CACHE["prog"] = nc
    return plan, nc


def kernel(u, a_rows, a_cols, bi_rows, bi_cols, c1_rows, c1_cols,
           c2_rows, c2_cols, out_int, **_ignored):
    from concourse.bass_utils import run_bass_kernel_spmd

    u = np.asarray(u)
    assert u.shape == (B_TOTAL, K)
    plan, nc = _get_plan_program(a_rows, a_cols, bi_rows, bi_cols,
                                 c1_rows, c1_cols, c2_rows, c2_cols)

    # ---- host marshalling: pack 16 batch rows per uint16 lane ----
    ub = u.astype(np.uint16)
    p128 = np.arange(128)
    lane = p128 % PL
    grp = p128 // PL
    in_maps = []
    for c in range(N_CORES):
        seg = ub[c * B_CORE:(c + 1) * B_CORE]          # [256, 8000]
        packed = np.zeros((PL, K), np.uint16)
        for t in range(PACK):
            packed |= (seg[t * PL:(t + 1) * PL] << t).astype(np.uint16)
        u_ext = np.concatenate([packed, np.zeros((PL, 1), np.uint16)], axis=1)
        uwc = u_ext[lane[:, None, None], plan.IDX8[grp]]   # [128, S, 384]
        in_maps.append({"uw": np.ascontiguousarray(
            uwc.reshape(128, plan.S_total * Z))})

    res = run_bass_kernel_spmd(nc, in_maps, core_ids=list(range(N_CORES)))

    # ---- host assembly ----
    oi = np.asarray(out_int)
    out = np.empty((B_TOTAL, N), np.float32)
    shift = np.arange(PACK, dtype=np.uint16)
    for c in range(N_CORES):
        opa = np.asarray(res.results[c]["opa"])        # [16, 1536]
        opb = np.asarray(res.results[c]["opb"])        # [128, NPOS*384]
        cs = np.empty((B_CORE, N), np.float32)
        cs[:, 0:K - 2 * Z] = u[c * B_CORE:(c + 1) * B_CORE, 2 * Z:K]
        pa_bits = ((opa[None, :, :] >> shift[:, None, None]) & 1)
        cs[:, K - 2 * Z:K - 2 * Z + PA_BITS] = (
            pa_bits.reshape(B_CORE, PA_BITS))
        pb = np.empty((B_CORE, PB * Z), np.float32)
        for r in range(PB):
            w = np.zeros((PL, Z), np.uint16)
            for (g_, p_, _, _) in plan.pieces[r]:
                w ^= opb[g_ * PL:(g_ + 1) * PL, p_ * Z:(p_ + 1) * Z]
            bits = ((w[None, :, :] >> shift[:, None, None]) & 1)
            pb[:, r * Z:(r + 1) * Z] = bits.reshape(B_CORE, Z)
        cs[:, K - 2 * Z + PA_BITS:] = pb[:, :PB_BITS]
        out[c * B_CORE:(c + 1) * B_CORE] = cs[:, oi]
    return out
